# revision 1
# baseline (speedup 1.0000x reference)
"""Multi-head attention (S=2048, B=2, D=1024, H=16) on 8 Trainium2 cores.

Sharding: tensor-parallel over heads (4 groups of 4 heads) x data-parallel
over batch (2). Core r handles batch r//4, heads [4*(r%4), 4*(r%4)+4).
Each core projects its 256 channels, runs attention for its 4 heads, applies
its slice of the output projection, and a ReduceScatter over each 4-core
batch group sums the partial outputs and leaves each core with a 512-row
slice of the final [2048, 1024] output.

All matmul operands are bf16 (fp32r measures ~2 cycles/column on this HW;
bf16 measures ~1), with fp32 PSUM accumulation. Softmax denominators come
free from an extra ones-column appended to V in the PV matmul. V's bias and
the output bias are folded out algebraically and added on the host.

The query blocks taper (512,512,512,256,128,128) so the trailing
ReduceScatter chunks shrink: the only collective that cannot overlap
compute is the final 128-row one.
"""
import sys

sys.path.insert(0, "/opt/trn_rl_repo")

import numpy as np
import ml_dtypes
import concourse.bacc as bacc
import concourse.mybir as mybir
from concourse import tile
from concourse.bass_utils import run_bass_kernel_spmd

dt = mybir.dt
AF = mybir.ActivationFunctionType
BF16 = ml_dtypes.bfloat16

S, B, D = 2048, 2, 1024
H, DK = 16, 64
NCORES = 8
HC = 4                 # heads per core
CH = HC * DK           # 256 local channels per core
SCALE = np.float32(1.0 / np.sqrt(DK))
GROUPS = [[0, 1, 2, 3], [4, 5, 6, 7]]

NKD = D // 128         # 8 contraction tiles for projections
NTK = S // 128         # 16 key tiles

# Tapered query blocks; each block is one ReduceScatter chunk.
TQS = [512, 512, 512, 512]
NB = len(TQS)
TQ0 = [sum(TQS[:i]) for i in range(NB)]
NSUB = [t // 128 for t in TQS]              # 128-row out subtiles per block
SUB0 = [t // 128 for t in TQ0]              # first subtile index of block


def build_nc():
    f32, bf16 = dt.float32, dt.bfloat16
    nc = bacc.Bacc("TRN2", target_bir_lowering=False, debug=False,
                   num_devices=NCORES)

    xq = nc.dram_tensor("xq_t", [D, S], bf16, kind="ExternalInput").ap()
    xk = nc.dram_tensor("xk_t", [D, S], bf16, kind="ExternalInput").ap()
    xv = nc.dram_tensor("xv_t", [D, S], bf16, kind="ExternalInput").ap()
    # all weights host-packed into one [128, 8196] tensor in SBUF layout:
    # [wk 2048 | wq 2048 | wv 2048 | wo 2048 | ones 4] - one 16KB-line DMA
    wpk = nc.dram_tensor("wpack", [128, 8196], bf16, kind="ExternalInput").ap()
    bq = nc.dram_tensor("bq", [2, 128], f32, kind="ExternalInput").ap()
    bk = nc.dram_tensor("bk", [2, 128], f32, kind="ExternalInput").ap()
    # Chunk c covers global token rows [TQ0[c], TQ0[c]+TQS[c]);
    # group-rank j receives rows TQ0[c] + j*TQS[c]//4 onward.
    out_ext = nc.dram_tensor("out_rs", [S // 4, D], bf16,
                             kind="ExternalOutput").ap()

    with tile.TileContext(nc) as tc:
        with tc.tile_pool(name="const", bufs=1) as cp, \
             tc.tile_pool(name="stream", bufs=1) as sp, \
             tc.tile_pool(name="psum", bufs=1, space="PSUM") as pp, \
             tc.tile_pool(name="dram", bufs=1, space="DRAM") as dp:

            # ---- resident weights: one packed tile, slice views ----
            bq_sb = [cp.tile([128, 1], f32, tag=f"bq{j}", name=f"bq{j}")
                     for j in range(2)]
            bk_sb = [cp.tile([128, 1], f32, tag=f"bk{j}", name=f"bk{j}")
                     for j in range(2)]
            for j in range(2):
                nc.scalar.dma_start(bk_sb[j][:], bk[j].unsqueeze(1))
                nc.scalar.dma_start(bq_sb[j][:], bq[j].unsqueeze(1))
            wpack_sb = cp.tile([128, 8196], bf16, tag="wpack", name="wpack_sb")
            nc.scalar.dma_start(wpack_sb[:], wpk[:])
            wk_sb = [wpack_sb[:, k * CH:(k + 1) * CH] for k in range(NKD)]
            wq_sb = [wpack_sb[:, 2048 + k * CH:2048 + (k + 1) * CH]
                     for k in range(NKD)]
            wv_sb = [wpack_sb[:, 4096 + k * CH:4096 + (k + 1) * CH]
                     for k in range(NKD)]
            wo_sb = [wpack_sb[:, 6144 + k * D:6144 + (k + 1) * D]
                     for k in range(2)]
            ones_sb = wpack_sb[:, 8192:8196]
            # xq half 0 also rides the scalar queue (ACT drains by ~20us,
            # before the first eviction is needed)
            xq_sb = {}
            for k in range(NKD):
                t_ = sp.tile([128, 1024], bf16, tag="xqr", bufs=16,
                             name=f"xqr0_{k}")
                nc.scalar.dma_start(t_[:], xq[k * 128:(k + 1) * 128, 0:1024])
                xq_sb[(0, k)] = t_

            # sync queue: xk halves, xq half 0, xv half 0, xq half 1.
            # gpsimd (software DGE) takes only xv half 1 (2MB, done long
            # before partition_broadcast is first needed).
            xk_sb = {}
            for hh in range(2):
                for k in range(NKD):
                    t_ = sp.tile([128, 1024], bf16, tag="xkr", bufs=16,
                                 name=f"xkr{hh}_{k}")
                    nc.sync.dma_start(
                        t_[:], xk[k * 128:(k + 1) * 128,
                                  hh * 1024:(hh + 1) * 1024])
                    xk_sb[(hh, k)] = t_
            xv_sb = {}
            for k in range(NKD):
                t_ = sp.tile([128, 1024], bf16, tag="xvr", bufs=16,
                             name=f"xvr0_{k}")
                nc.sync.dma_start(t_[:], xv[k * 128:(k + 1) * 128, 0:1024])
                xv_sb[(0, k)] = t_
            for k in range(NKD):
                t_ = sp.tile([128, 1024], bf16, tag="xvr", bufs=16,
                             name=f"xvr1_{k}")
                nc.gpsimd.dma_start(t_[:],
                                    xv[k * 128:(k + 1) * 128, 1024:2048])
                xv_sb[(1, k)] = t_
            for k in range(NKD):
                t_ = sp.tile([128, 1024], bf16, tag="xqr", bufs=16,
                             name=f"xqr1_{k}")
                nc.sync.dma_start(t_[:], xq[k * 128:(k + 1) * 128, 1024:2048])
                xq_sb[(1, k)] = t_

            # ---- persistent activations ----
            qc = [cp.tile([128, S], bf16, tag=f"qc{j}", name=f"qc{j}")
                  for j in range(2)]
            kc = [cp.tile([128, S], bf16, tag=f"kc{j}", name=f"kc{j}")
                  for j in range(2)]
            # V tiles: [token128, 4*(64 V + 1 ones)] per key tile
            vt = [cp.tile([128, HC * (DK + 1)], bf16, tag=f"vt{t}",
                          name=f"vt{t}") for t in range(NTK)]
            ctx = [cp.tile([128, S], bf16, tag=f"ctx{j}", name=f"ctx{j}")
                   for j in range(2)]
            for t in range(NTK):
                vt_view = vt[t][:].rearrange("p (h c) -> p h c", h=HC)
                nc.vector.tensor_copy(vt_view[:, :, DK:DK + 1],
                                      ones_sb.unsqueeze(2))

            # ---- projections: K first (PE never in-order-blocks on the
            # later weights), then Q tokens 0:1024, V, Q tokens 1024:2048.
            TP = 512

            def kproj_quarter(th):
                for j in range(2):
                    ps = pp.tile([128, TP], f32, tag="cx", bufs=2,
                                 name=f"kp{th}_{j}")
                    for k in range(NKD):
                        nc.tensor.matmul(
                            ps[:], wk_sb[k][:, j * 128:(j + 1) * 128],
                            xk_sb[(th // 2, k)][:, (th % 2) * TP:
                                                (th % 2) * TP + TP],
                            start=(k == 0), stop=(k == NKD - 1))
                    nc.scalar.activation(
                        kc[j][:, th * TP:(th + 1) * TP], ps[:],
                        AF.Identity, bias=bk_sb[j][:, 0:1])

            def qproj_half(th):
                for t2 in range(2):
                    tq0 = th * 1024 + t2 * TP
                    for j in range(2):
                        ps = pp.tile([128, TP], f32, tag="cx", bufs=2,
                                     name=f"qp{th}_{j}")
                        for k in range(NKD):
                            nc.tensor.matmul(
                                ps[:], wq_sb[k][:, j * 128:(j + 1) * 128],
                                xq_sb[(th, k)][:, t2 * TP:(t2 + 1) * TP],
                                start=(k == 0), stop=(k == NKD - 1))
                        nc.scalar.activation(
                            qc[j][:, tq0:tq0 + TP], ps[:],
                            AF.Identity, bias=bq_sb[j][:, 0:1])

            kproj_quarter(0)
            kproj_quarter(1)
            qproj_half(0)
            kproj_quarter(2)
            kproj_quarter(3)

            # ---- V projection, tiles 0..7 as a phase; tiles 8..15 are
            # woven one-per-step into block 0's first attention steps ----
            for t in range(NTK // 2):
                pv = pp.tile([128, CH], f32, tag="cx", bufs=2,
                             name=f"pv{t}")
                for k in range(NKD):
                    nc.tensor.matmul(
                        pv[:], xv_sb[(t // 8, k)][:, (t % 8) * 128:
                                                  (t % 8) * 128 + 128],
                        wv_sb[k],
                        start=(k == 0), stop=(k == NKD - 1))
                # strided copy into [128, 4, 65][:, :, 0:64]
                dst_view = vt[t][:].rearrange("p (h c) -> p h c", h=HC)
                src_view = pv[:].rearrange("p (h c) -> p h c", h=HC)
                nc.vector.tensor_copy(dst_view[:, :, 0:DK], src_view)

            qproj_half(1)

            def vproj_tile(t):
                pv = pp.tile([128, 1024], f32, tag="s1", bufs=3,
                             name=f"pv{t}")
                for k in range(NKD):
                    nc.tensor.matmul(
                        pv[:, 0:CH],
                        xv_sb[(t // 8, k)][:, (t % 8) * 128:
                                           (t % 8) * 128 + 128],
                        wv_sb[k],
                        start=(k == 0), stop=(k == NKD - 1))
                dst_view = vt[t][:].rearrange("p (h c) -> p h c", h=HC)
                src_view = pv[:, 0:CH].rearrange("p (h c) -> p h c", h=HC)
                nc.vector.tensor_copy(dst_view[:, :, 0:DK], src_view)

            # ---- attention + output projection ----
            # Warmup collective: the first CC op on the stream pays a
            # ~15-25us warmup penalty; absorb it with a tiny dummy
            # ReduceScatter (contents irrelevant, output unused) issued at
            # kernel start so the real per-chunk ops run at steady state.
            cc_ins = [dp.tile([TQS[c], D], bf16, tag=f"ccin{c}",
                              name=f"cc_in{c}") for c in range(NB)]
            cc_warm_in = dp.tile([16, D], bf16, tag="ccwi", name="cc_warm_in")
            cc_warm_out = dp.tile([4, D], bf16, tag="ccwo",
                                  name="cc_warm_out")
            nc.gpsimd.collective_compute(
                "ReduceScatter", mybir.AluOpType.add,
                replica_groups=GROUPS,
                ins=[cc_warm_in[:]], outs=[cc_warm_out[:]])
            cc_outs = [dp.tile([TQS[c] // 4, D], bf16, tag=f"ccout{c}",
                               name=f"cc_out{c}") for c in range(NB)]

            def emit_outproj_subtile(sub, chunk):
                """Out-projection + store for one 128-row output subtile."""
                t0 = sub * 128
                po = pp.tile([128, 1024], f32, tag="s1", bufs=3,
                             name=f"po{sub}")
                for e in range(2):
                    for dv in range(2):
                        nc.tensor.matmul(
                            po[:, e * 512:(e + 1) * 512],
                            ctx[dv][:, t0:t0 + 128],
                            wo_sb[dv][:, e * 512:(e + 1) * 512],
                            start=(dv == 0), stop=(dv == 1))
                osb = sp.tile([128, D], bf16, tag="ot", bufs=8,
                              name=f"ot{sub}")
                nc.vector.tensor_copy(osb[:], po[:])
                r0 = sub * 128 - TQ0[chunk]
                nc.sync.dma_start(cc_ins[chunk][r0:r0 + 128, :], osb[:])
                if sub + 1 == SUB0[chunk] + NSUB[chunk]:
                    # chunk complete: ReduceScatter it (overlaps the
                    # attention compute of the following blocks)
                    nc.gpsimd.collective_compute(
                        "ReduceScatter", mybir.AluOpType.add,
                        replica_groups=GROUPS,
                        ins=[cc_ins[chunk][:]], outs=[cc_outs[chunk][:]])

            # Flattened attention stream over all (block, head-pair)
            # steps, with each PV pair deferred PV_LAG steps behind its
            # scores/exp. The deferral crosses pair boundaries, so the
            # next pair's scores+exp issue before the previous pair's
            # last PV and its cx-PSUM evacuation latency stays off the
            # ACT engine's critical path.
            PV_LAG = 2

            def emit_normalize(bi, p, cx):
                tq0, tqn = TQ0[bi], TQS[bi]
                cxs = []
                for h in range(2):
                    c_ = sp.tile([65, 512], f32, tag="cxs", bufs=4,
                                 name=f"cxs{p}_{h}")
                    nc.vector.tensor_copy(c_[:, 0:tqn], cx[h][:])
                    cxs.append(c_[:, 0:tqn])
                for h in range(2):
                    den = sp.tile([1, 512], f32, tag="den", bufs=2,
                                  name=f"den{p}_{h}")
                    nc.vector.tensor_copy(den[:, 0:tqn], cxs[h][64:65, :])
                    rc = sp.tile([1, 512], f32, tag="rc", bufs=2,
                                 name=f"rc{p}_{h}")
                    nc.vector.reciprocal_approx_fast(rc[:, 0:tqn],
                                                     den[:, 0:tqn])
                    bc = sp.tile([64, 512], f32, tag="bc", bufs=4,
                                 name=f"bc{p}_{h}")
                    nc.gpsimd.partition_broadcast(bc[:, 0:tqn],
                                                  rc[:, 0:tqn])
                    nc.vector.tensor_mul(
                        ctx[p][h * 64:(h + 1) * 64, tq0:tq0 + tqn],
                        cxs[h][0:64, :], bc[:, 0:tqn])

            pvq = []        # deferred PV steps: (bi, p, tk, etf, cx)

            def pop_pv():
                bi_, p_, tk_, etf_, cx_ = pvq.pop(0)
                tqn_ = TQS[bi_]
                for h in range(2):
                    hl = p_ * 2 + h
                    nc.tensor.matmul(
                        cx_[h][:],
                        vt[tk_][:, hl * 65:(hl + 1) * 65],
                        etf_[:, h * 512:h * 512 + tqn_],
                        start=(tk_ == 0), stop=(tk_ == NTK - 1))
                if tk_ == NTK - 1:
                    emit_normalize(bi_, p_, cx_)

            for bi in range(NB):
                tq0, tqn = TQ0[bi], TQS[bi]
                for p in range(2):             # head pairs (2p, 2p+1)
                    cxf = [pp.tile([65, 512], f32, tag="cx", bufs=2,
                                   name=f"cx{p}_{h}") for h in range(2)]
                    cx = [c_[:, 0:tqn] for c_ in cxf]
                    for tk in range(NTK):
                        if bi == 0 and p == 0 and tk < NTK // 2:
                            vproj_tile(tk + 8)
                        # previous block's out-projection, interleaved a few
                        # steps into this block so its ctx (behind the
                        # normalize chain) is ready when the PE reaches it
                        if bi > 0 and p == 0 and tk >= 4 and \
                                (tk - 4) % 3 == 0:
                            j = (tk - 4) // 3
                            if j < NSUB[bi - 1]:
                                emit_outproj_subtile(SUB0[bi - 1] + j, bi - 1)
                        # head h at col offset 512*h: every matmul PSUM
                        # output starts on a 2KB bank boundary
                        s1f = pp.tile([128, 1024], f32, tag="s1", bufs=3,
                                      name=f"s1{tk}")
                        etf = sp.tile([128, 1024], bf16, tag="et", bufs=8,
                                      name=f"et{tk}")
                        for h in range(2):      # adjacent -> row-pack overlap
                            r0 = h * 64
                            nc.tensor.matmul(
                                s1f[:, h * 512:h * 512 + tqn],
                                kc[p][r0:r0 + 64, tk * 128:(tk + 1) * 128],
                                qc[p][r0:r0 + 64, tq0:tq0 + tqn],
                                start=True, stop=True)
                        if tqn == 512:
                            nc.scalar.activation(etf[:], s1f[:], AF.Exp)
                        else:
                            for h in range(2):
                                nc.scalar.activation(
                                    etf[:, h * 512:h * 512 + tqn],
                                    s1f[:, h * 512:h * 512 + tqn], AF.Exp)
                        pvq.append((bi, p, tk, etf, cx))
                        while len(pvq) > PV_LAG:
                            pop_pv()
            while pvq:
                pop_pv()
            # last block's out-projection
            for j in range(NSUB[NB - 1]):
                emit_outproj_subtile(SUB0[NB - 1] + j, NB - 1)

            # final stores, force-scheduled at the very end so a store
            # waiting on its ReduceScatter never head-of-line-blocks the
            # sync DMA queue mid-kernel
            with tc.tile_wait_until(10):
                for c in range(NB):
                    o0 = TQ0[c] // 4
                    nc.sync.dma_start(out_ext[o0:o0 + TQS[c] // 4, :],
                                      cc_outs[c][:])

    nc.finalize()
    return nc


_NC = None


def _get_nc():
    global _NC
    if _NC is None:
        _NC = build_nc()
    return _NC


def make_in_maps(q, k, v, Wq, bq, Wk, bk, Wv, bv, Wo, bo):
    """Shard + precondition full inputs into per-core input maps."""
    xq_b = [np.ascontiguousarray(q[:, b, :].T).astype(BF16) for b in range(B)]
    xk_b = [np.ascontiguousarray(k[:, b, :].T).astype(BF16) for b in range(B)]
    xv_b = [np.ascontiguousarray(v[:, b, :].T).astype(BF16) for b in range(B)]
    in_maps = []
    for r in range(NCORES):
        b = r // 4
        g = r % 4
        ch = slice(g * CH, (g + 1) * CH)
        def pack_dk(wt, width):
            # [1024 or 256 rows, width] -> [128, nk*width] k-slices side by side
            nk = wt.shape[0] // 128
            return wt.reshape(nk, 128, width).transpose(1, 0, 2).reshape(
                128, nk * width)
        wk_t = np.ascontiguousarray(Wk[ch, :].T).astype(BF16)
        wq_t = np.ascontiguousarray((Wq[ch, :] * SCALE).T).astype(BF16)
        wv_t = np.ascontiguousarray(Wv[ch, :].T).astype(BF16)
        wo_t = np.ascontiguousarray(Wo[:, ch].T).astype(BF16)
        wpack = np.concatenate([
            pack_dk(wk_t, CH), pack_dk(wq_t, CH), pack_dk(wv_t, CH),
            pack_dk(wo_t, D), np.ones((128, HC), dtype=BF16)], axis=1)
        in_maps.append({
            "xq_t": xq_b[b], "xk_t": xk_b[b], "xv_t": xv_b[b],
            "wpack": np.ascontiguousarray(wpack),
            "bq": (bq[ch] * SCALE).reshape(2, 128).astype(np.float32),
            "bk": bk[ch].reshape(2, 128).astype(np.float32),
        })
    return in_maps


def assemble(results, Wo, bv, bo):
    """Gather per-core ReduceScatter slices into the full [S, B, D] output."""
    out = np.empty((S, B, D), dtype=np.float32)
    for r in range(NCORES):
        b = r // 4
        j = r % 4
        for c in range(NB):
            rows = TQS[c] // 4
            g0 = TQ0[c] + j * rows               # global token rows
            o0 = TQ0[c] // 4                     # rows within out_rs
            out[g0:g0 + rows, b, :] = \
                results[r]["out_rs"][o0:o0 + rows].astype(np.float32)
    out += (bo + Wo @ bv).astype(np.float32)
    return out


def run_sharded(inputs, trace=False):
    nc = _get_nc()
    in_maps = make_in_maps(**inputs)
    res = run_bass_kernel_spmd(nc, in_maps, list(range(NCORES)), trace=trace)
    full = assemble(res.results, np.asarray(inputs["Wo"], dtype=np.float32),
                    np.asarray(inputs["bv"], dtype=np.float32),
                    np.asarray(inputs["bo"], dtype=np.float32))
    return full, res


def kernel(**inputs) -> np.ndarray:
    inputs = {k_: np.asarray(v_, dtype=np.float32)
              for k_, v_ in inputs.items()}
    full, _ = run_sharded(inputs)
    return full



# revision 9
# speedup vs baseline: 1.1619x; 1.1619x over previous
"""Multi-head attention (S=2048, B=2, D=1024, H=16) on 8 Trainium2 cores.

Sharding: tensor-parallel over heads (4 groups of 4 heads) x data-parallel
over batch (2). Core r handles batch r//4, heads [4*(r%4), 4*(r%4)+4).

Structure (v2): the scalar engine's 128 Exp activations (~1.35us each) are
the roofline; everything else is arranged around keeping that stream dense:
  - minimal prefix (one K-proj quarter + one Q-proj quarter), all other
    projections and the output projection woven into attention steps;
  - Q bias folded in as a 9th K=1 matmul (ones row x bias row); K bias
    dropped entirely (softmax is invariant to per-query constants), so the
    scalar engine runs Exp only;
  - inputs arrive as 1MB quarter-DMAs in consumption order;
  - ReduceScatter split into 7 chunks (6x256 + 512 rows) so mid-stream RS
    ops (~16us) hide under compute and only the last (~23us) is exposed.

All matmul operands bf16, fp32 PSUM accumulation. Softmax denominators come
free from a ones-column appended to V. V's bias and the output bias are
folded out algebraically and added on the host.
"""
import sys

sys.path.insert(0, "/opt/trn_rl_repo")

import numpy as np
import ml_dtypes
import concourse.bacc as bacc
import concourse.mybir as mybir
from concourse import tile
from concourse.bass_utils import run_bass_kernel_spmd

dt = mybir.dt
AF = mybir.ActivationFunctionType
BF16 = ml_dtypes.bfloat16

S, B, D = 2048, 2, 1024
H, DK = 16, 64
NCORES = 8
HC = 4                 # heads per core
CH = HC * DK           # 256 local channels per core
SCALE = np.float32(1.0 / np.sqrt(DK))
GROUPS = [[0, 1, 2, 3], [4, 5, 6, 7]]

NKD = D // 128         # 8 contraction tiles for projections
NTK = S // 128         # 16 key tiles
NB = 4                 # attention blocks of 512 q tokens
PV_LAG = 5

# wpack column offsets: [wk 2048 | ones 4 | wq 2048 | wv 2048 | wo 2048]
WK0, ON0, WQ0, WV0, WO0 = 0, 2048, 2052, 4100, 6148
WPCOLS = 8196

# RS chunks over output rows: 6 x 256 + 1 x 512 (subtiles are 128 rows)
CHUNK_SUBS = [[0, 1], [2, 3], [4, 5], [6, 7], [8, 9], [10, 11],
              [12, 13, 14, 15]]
NCH = len(CHUNK_SUBS)
CHROW0 = [c[0] * 128 for c in CHUNK_SUBS]
CHROWS = [len(c) * 128 for c in CHUNK_SUBS]


def build_nc():
    f32, bf16 = dt.float32, dt.bfloat16
    nc = bacc.Bacc("TRN2", target_bir_lowering=False, debug=False,
                   num_devices=NCORES)

    # inputs: [128, 16384] with layout (p, quarter, k, t) per tensor
    xq = nc.dram_tensor("xq_t", [128, 16384], bf16, kind="ExternalInput").ap()
    xk = nc.dram_tensor("xk_t", [128, 16384], bf16, kind="ExternalInput").ap()
    xv = nc.dram_tensor("xv_t", [128, 16384], bf16, kind="ExternalInput").ap()
    wpk = nc.dram_tensor("wpack", [128, WPCOLS], bf16,
                         kind="ExternalInput").ap()
    # [1, 768]: [bq j0 128 | bq j1 128 | ones 512]
    bqo = nc.dram_tensor("bqo", [1, 768], bf16, kind="ExternalInput").ap()
    out_ext = nc.dram_tensor("out_rs", [S // 4, D], bf16,
                             kind="ExternalOutput").ap()

    with tile.TileContext(nc) as tc:
        with tc.tile_pool(name="const", bufs=1) as cp, \
             tc.tile_pool(name="stream", bufs=1) as sp, \
             tc.tile_pool(name="psum", bufs=1, space="PSUM") as pp, \
             tc.tile_pool(name="dram", bufs=1, space="DRAM") as dp:

            # ---- CC warmup: absorb the collective stream's init cost ----
            cc_warm_in = dp.tile([16, D], bf16, tag="ccwi", name="cc_warm_in")
            cc_warm_out = dp.tile([4, D], bf16, tag="ccwo",
                                  name="cc_warm_out")
            nc.gpsimd.collective_compute(
                "ReduceScatter", mybir.AluOpType.add,
                replica_groups=GROUPS,
                ins=[cc_warm_in[:]], outs=[cc_warm_out[:]])

            # ---- weights / bias ----
            bqo_sb = cp.tile([1, 768], bf16, tag="bqo", name="bqo_sb")
            nc.scalar.dma_start(bqo_sb[:], bqo[:])
            wpack_sb = cp.tile([128, WPCOLS], bf16, tag="wpack",
                               name="wpack_sb")
            nc.scalar.dma_start(wpack_sb[:, WK0:WQ0], wpk[:, WK0:WQ0])
            nc.scalar.dma_start(wpack_sb[:, WQ0:WV0], wpk[:, WQ0:WV0])
            nc.scalar.dma_start(wpack_sb[:, WV0:WO0], wpk[:, WV0:WO0])
            wk_sb = [wpack_sb[:, WK0 + k * CH:WK0 + (k + 1) * CH]
                     for k in range(NKD)]
            wq_sb = [wpack_sb[:, WQ0 + k * CH:WQ0 + (k + 1) * CH]
                     for k in range(NKD)]
            wv_sb = [wpack_sb[:, WV0 + k * CH:WV0 + (k + 1) * CH]
                     for k in range(NKD)]
            wo_sb = [wpack_sb[:, WO0 + k * D:WO0 + (k + 1) * D]
                     for k in range(2)]
            ones_sb = wpack_sb[:, ON0:ON0 + 4]

            # exp table preload so the first real Exp doesn't pay ~2.7us
            pre_sb = cp.tile([1, 16], f32, tag="pre", name="pre_sb")
            nc.vector.memset(pre_sb[:], 0.0)
            pre_o = cp.tile([1, 16], f32, tag="preo", name="preo_sb")
            nc.scalar.activation(pre_o[:], pre_sb[:], AF.Exp)

            # ---- input quarters (1MB DMAs, consumption order) ----
            # sync queue: xk0 xq0 xk1 xv0 xk2 xv1 xk3 xv2 xv3
            xkq, xqq, xvq = {}, {}, {}
            _tags = {id(None): "_"}

            def load_q(dst_map, tagc, src, qi, eng):
                t_ = sp.tile([128, 4096], bf16, tag=f"x{tagc}",
                             bufs=4, name=f"x{tagc}{qi}")
                eng.dma_start(t_[:], src[:, qi * 4096:(qi + 1) * 4096])
                dst_map[qi] = t_

            load_q(xkq, "k", xk, 0, nc.sync)
            load_q(xvq, "v", xv, 0, nc.sync)
            load_q(xkq, "k", xk, 1, nc.sync)
            load_q(xvq, "v", xv, 1, nc.sync)
            load_q(xkq, "k", xk, 2, nc.sync)
            load_q(xvq, "v", xv, 2, nc.sync)
            load_q(xkq, "k", xk, 3, nc.sync)
            load_q(xvq, "v", xv, 3, nc.sync)
            # gpsimd ring carries xq0 so it lands in parallel with xk0;
            # xq1-3/wo deferred into the weave so they don't compete with
            # the critical front-window loads
            load_q(xqq, "q", xq, 0, nc.gpsimd)

            def xslice(m, qi, k):
                return m[qi][:, k * 512:(k + 1) * 512]

            # ---- persistent activations ----
            qc = [cp.tile([128, S], bf16, tag=f"qc{j}", name=f"qc{j}")
                  for j in range(2)]
            kc = [cp.tile([128, S], bf16, tag=f"kc{j}", name=f"kc{j}")
                  for j in range(2)]
            vt = [cp.tile([128, HC * (DK + 1)], bf16, tag=f"vt{t}",
                          name=f"vt{t}") for t in range(NTK)]
            ctx = [cp.tile([128, S], bf16, tag=f"ctx{j}", name=f"ctx{j}")
                   for j in range(2)]
            for t in range(NTK):
                vt_view = vt[t][:].rearrange("p (h c) -> p h c", h=HC)
                nc.vector.tensor_copy(vt_view[:, :, DK:DK + 1],
                                      ones_sb.unsqueeze(2))

            # ---- projection emitters (psum tag "pj", 1 bank) ----
            def kproj(th, j):
                ps = pp.tile([128, 512], f32, tag="pj", bufs=2,
                             name=f"kp{th}_{j}")
                for k in range(NKD):
                    nc.tensor.matmul(
                        ps[:], wk_sb[k][:, j * 128:(j + 1) * 128],
                        xslice(xkq, th, k),
                        start=(k == 0), stop=(k == NKD - 1))
                nc.vector.tensor_copy(kc[j][:, th * 512:(th + 1) * 512],
                                      ps[:])

            def qproj(th, j):
                ps = pp.tile([128, 512], f32, tag="pj", bufs=2,
                             name=f"qp{th}_{j}")
                for k in range(NKD):
                    nc.tensor.matmul(
                        ps[:], wq_sb[k][:, j * 128:(j + 1) * 128],
                        xslice(xqq, th, k),
                        start=(k == 0), stop=False)
                # 9th matmul adds bq via ones row (K=1)
                nc.tensor.matmul(
                    ps[:], bqo_sb[0:1, j * 128:(j + 1) * 128],
                    bqo_sb[0:1, 256:768],
                    start=False, stop=True)
                nc.vector.tensor_copy(qc[j][:, th * 512:(th + 1) * 512],
                                      ps[:])

            def vproj(t, p):
                # V projection for key tile t, head pair p (128 channels)
                ps = pp.tile([128, 512], f32, tag="pj", bufs=2,
                             name=f"vp{t}_{p}")
                for k in range(NKD):
                    nc.tensor.matmul(
                        ps[:, 0:128],
                        xslice(xvq, t // 4, k)[:, (t % 4) * 128:
                                               (t % 4) * 128 + 128],
                        wv_sb[k][:, p * 128:(p + 1) * 128],
                        start=(k == 0), stop=(k == NKD - 1))
                dst = vt[t][:, p * 130:(p + 1) * 130].rearrange(
                    "p (h c) -> p h c", h=2)
                src = ps[:, 0:128].rearrange("p (h c) -> p h c", h=2)
                nc.vector.tensor_copy(dst[:, :, 0:DK], src)

            # ---- collective buffers ----
            cc_ins = [dp.tile([CHROWS[c], D], bf16, tag=f"ccin{c}",
                              name=f"cc_in{c}") for c in range(NCH)]
            cc_outs = [dp.tile([CHROWS[c] // 4, D], bf16, tag=f"ccout{c}",
                               name=f"cc_out{c}") for c in range(NCH)]

            def outproj_subtile(sub, tail=False):
                t0 = sub * 128
                osb = sp.tile([128, D], bf16, tag="ot", bufs=4,
                              name=f"ot{sub}")
                if tail:
                    # scores are done; reuse the wide s1 psum ring so the
                    # four matmuls pipeline with a single evacuation copy
                    po = pp.tile([128, 1024], f32, tag="s1", bufs=2,
                                 name=f"pot{sub}")
                    for e in range(2):
                        for dv in range(2):
                            nc.tensor.matmul(
                                po[:, e * 512:(e + 1) * 512],
                                ctx[dv][:, t0:t0 + 128],
                                wo_sb[dv][:, e * 512:(e + 1) * 512],
                                start=(dv == 0), stop=(dv == 1))
                    nc.vector.tensor_copy(osb[:], po[:])
                else:
                    for e in range(2):
                        po = pp.tile([128, 512], f32, tag="pj", bufs=2,
                                     name=f"po{sub}_{e}")
                        for dv in range(2):
                            nc.tensor.matmul(
                                po[:], ctx[dv][:, t0:t0 + 128],
                                wo_sb[dv][:, e * 512:(e + 1) * 512],
                                start=(dv == 0), stop=(dv == 1))
                        nc.vector.tensor_copy(osb[:, e * 512:(e + 1) * 512],
                                              po[:])
                c = next(i for i, subs in enumerate(CHUNK_SUBS)
                         if sub in subs)
                r0 = t0 - CHROW0[c]
                nc.sync.dma_start(cc_ins[c][r0:r0 + 128, :], osb[:])

            def chunk_rs(c):
                nc.gpsimd.collective_compute(
                    "ReduceScatter", mybir.AluOpType.add,
                    replica_groups=GROUPS,
                    ins=[cc_ins[c][:]], outs=[cc_outs[c][:]])

            # ---- normalize (per pair, per block) ----
            def emit_normalize(bi, p, cx):
                tq0 = bi * 512
                cxs = []
                for h in range(2):
                    c_ = sp.tile([65, 512], f32, tag="cxs", bufs=4,
                                 name=f"cxs{p}_{h}")
                    nc.vector.tensor_copy(c_[:], cx[h][:])
                    cxs.append(c_)
                for h in range(2):
                    den = sp.tile([1, 512], f32, tag="den", bufs=2,
                                  name=f"den{p}_{h}")
                    nc.vector.tensor_copy(den[:], cxs[h][64:65, :])
                    rc = sp.tile([1, 512], f32, tag="rc", bufs=2,
                                 name=f"rc{p}_{h}")
                    nc.vector.reciprocal_approx_fast(rc[:], den[:])
                    bc = sp.tile([64, 512], f32, tag="bc", bufs=4,
                                 name=f"bc{p}_{h}")
                    nc.gpsimd.partition_broadcast(bc[:], rc[:])
                    nc.vector.tensor_mul(
                        ctx[p][h * 64:(h + 1) * 64, tq0:tq0 + 512],
                        cxs[h][0:64, :], bc[:])

            # ---- deferred-PV machinery ----
            pvq = []

            def pop_pv():
                bi_, p_, tk_, etf_, cx_ = pvq.pop(0)
                for h in range(2):
                    hl = p_ * 2 + h
                    nc.tensor.matmul(
                        cx_[h][:],
                        vt[tk_][:, hl * 65:(hl + 1) * 65],
                        etf_[:, h * 512:(h + 1) * 512],
                        start=(tk_ == 0), stop=(tk_ == NTK - 1))
                if tk_ == NTK - 1:
                    emit_normalize(bi_, p_, cx_)

            # ---- weave table: step -> tasks ----
            weave = {}

            def wv_add(s, fn):
                weave.setdefault(s, []).append(fn)

            for t in range(16):
                wv_add(t + 3, lambda t=t: vproj(t, 0))
                wv_add(18 + t, lambda t=t: vproj(t, 1))
            wv_add(2, lambda: kproj(1, 0))
            wv_add(5, lambda: kproj(2, 0))
            wv_add(8, lambda: load_q(xqq, "q", xq, 1, nc.gpsimd))
            wv_add(8, lambda: kproj(3, 0))
            wv_add(12, lambda: qproj(0, 1))
            wv_add(14, lambda: kproj(0, 1))
            wv_add(16, lambda: kproj(1, 1))
            wv_add(20, lambda: kproj(2, 1))
            wv_add(24, lambda: kproj(3, 1))
            wv_add(17, lambda: nc.gpsimd.dma_start(
                wpack_sb[:, WO0:WPCOLS], wpk[:, WO0:WPCOLS]))
            wv_add(20, lambda: load_q(xqq, "q", xq, 2, nc.gpsimd))
            wv_add(26, lambda: qproj(1, 0))
            wv_add(28, lambda: qproj(1, 1))
            wv_add(52, lambda: load_q(xqq, "q", xq, 3, nc.gpsimd))
            wv_add(54, lambda: qproj(2, 0))
            wv_add(58, lambda: qproj(2, 1))
            wv_add(86, lambda: qproj(3, 0))
            wv_add(90, lambda: qproj(3, 1))
            # outproj of block bi-1 woven into block bi (subtiles every 3
            # steps from step 6); chunk RS after its last subtile
            for bi in range(1, NB):
                for i, sub in enumerate(range((bi - 1) * 4, bi * 4)):
                    s = bi * 32 + 6 + 3 * i
                    wv_add(s, lambda sub=sub: outproj_subtile(sub))
                    c = next(ci for ci, subs in enumerate(CHUNK_SUBS)
                             if sub in subs)
                    if sub == CHUNK_SUBS[c][-1]:
                        wv_add(s, lambda c=c: chunk_rs(c))

            # ---- prefix compute ----
            kproj(0, 0)
            qproj(0, 0)

            # ---- attention stream ----
            cx_cur = None
            for bi in range(NB):
                tq0 = bi * 512
                for p in range(2):
                    cx_cur = [pp.tile([65, 512], f32, tag="cx", bufs=2,
                                      name=f"cx{p}_{h}") for h in range(2)]
                    for tk in range(NTK):
                        s = bi * 32 + p * 16 + tk
                        s1 = pp.tile([128, 1024], f32, tag="s1", bufs=2,
                                     name=f"s1_{tk}")
                        etf = sp.tile([128, 1024], bf16, tag="et", bufs=8,
                                      name=f"et{tk}")
                        for h in range(2):
                            r0 = h * 64
                            nc.tensor.matmul(
                                s1[:, h * 512:(h + 1) * 512],
                                kc[p][r0:r0 + 64, tk * 128:(tk + 1) * 128],
                                qc[p][r0:r0 + 64, tq0:tq0 + 512],
                                start=True, stop=True)
                        nc.scalar.activation(etf[:], s1[:], AF.Exp)
                        pvq.append((bi, p, tk, etf, cx_cur))
                        while len(pvq) > PV_LAG:
                            pop_pv()
                        for fn in weave.get(s, []):
                            fn()
            while pvq:
                pop_pv()
            # last block's outproj + final chunk RS
            for sub in range(12, 16):
                outproj_subtile(sub, tail=True)
            chunk_rs(NCH - 1)

            # final stores, scheduled at the very end
            with tc.tile_wait_until(10):
                for c in range(NCH):
                    o0 = CHROW0[c] // 4
                    eng = nc.sync if c % 2 == 0 else nc.gpsimd
                    eng.dma_start(out_ext[o0:o0 + CHROWS[c] // 4, :],
                                  cc_outs[c][:])

    nc.finalize()
    return nc


_NC = None


def _get_nc():
    global _NC
    if _NC is None:
        _NC = build_nc()
    return _NC


def _pack_x(xb):
    # [1024, 2048] (d, s) -> [128, 16384] layout (p, quarter, k, t)
    return np.ascontiguousarray(
        xb.reshape(NKD, 128, 4, 512).transpose(1, 2, 0, 3).reshape(
            128, 16384)).astype(BF16)


def make_in_maps(q, k, v, Wq, bq, Wk, bk, Wv, bv, Wo, bo):
    """Shard + precondition full inputs into per-core input maps."""
    xq_b = [_pack_x(np.asarray(q[:, b, :].T, dtype=np.float32))
            for b in range(B)]
    xk_b = [_pack_x(np.asarray(k[:, b, :].T, dtype=np.float32))
            for b in range(B)]
    xv_b = [_pack_x(np.asarray(v[:, b, :].T, dtype=np.float32))
            for b in range(B)]
    in_maps = []
    for r in range(NCORES):
        b = r // 4
        g = r % 4
        ch = slice(g * CH, (g + 1) * CH)

        def pack_dk(wt, width):
            nk = wt.shape[0] // 128
            return wt.reshape(nk, 128, width).transpose(1, 0, 2).reshape(
                128, nk * width)

        wk_t = np.ascontiguousarray(Wk[ch, :].T).astype(BF16)
        wq_t = np.ascontiguousarray((Wq[ch, :] * SCALE).T).astype(BF16)
        wv_t = np.ascontiguousarray(Wv[ch, :].T).astype(BF16)
        wo_t = np.ascontiguousarray(Wo[:, ch].T).astype(BF16)
        wpack = np.concatenate([
            pack_dk(wk_t, CH), np.ones((128, 4), dtype=BF16),
            pack_dk(wq_t, CH), pack_dk(wv_t, CH),
            pack_dk(wo_t, D)], axis=1)
        bqo = np.concatenate([
            (np.asarray(bq[ch], dtype=np.float32) * SCALE).astype(BF16),
            np.ones((512,), dtype=BF16)]).reshape(1, 768)
        in_maps.append({
            "xq_t": xq_b[b], "xk_t": xk_b[b], "xv_t": xv_b[b],
            "wpack": np.ascontiguousarray(wpack),
            "bqo": np.ascontiguousarray(bqo),
        })
    return in_maps


def assemble(results, Wo, bv, bo):
    """Gather per-core RS slices into the full [S, B, D] output."""
    out = np.empty((S, B, D), dtype=np.float32)
    for r in range(NCORES):
        b = r // 4
        j = r % 4
        for c in range(NCH):
            rows = CHROWS[c] // 4
            g0 = CHROW0[c] + j * rows        # global token rows
            o0 = CHROW0[c] // 4              # rows within out_rs
            out[g0:g0 + rows, b, :] = \
                results[r]["out_rs"][o0:o0 + rows].astype(np.float32)
    out += (bo + Wo @ bv).astype(np.float32)
    return out


def run_sharded(inputs, trace=False):
    nc = _get_nc()
    in_maps = make_in_maps(**inputs)
    res = run_bass_kernel_spmd(nc, in_maps, list(range(NCORES)), trace=trace)
    full = assemble(res.results, np.asarray(inputs["Wo"], dtype=np.float32),
                    np.asarray(inputs["bv"], dtype=np.float32),
                    np.asarray(inputs["bo"], dtype=np.float32))
    return full, res


def kernel(**inputs) -> np.ndarray:
    inputs = {k_: np.asarray(v_, dtype=np.float32)
              for k_, v_ in inputs.items()}
    full, _ = run_sharded(inputs)
    return full
